# revision 2
# baseline (speedup 1.0000x reference)
"""nn_Network1 SLAYER-style spiking CNN, fully on 8 trn2 NeuronCores.

Sharding: pure data parallel, batch B=8 -> 1 sample per core. The ENTIRE
network (temporal SRM FIRs, spatial convs, sequential LIF spike scans
with refractory kernels, transpose-conv + bilinear upsample) runs in ONE
Bass program per core; the host only packs inputs (spikes as uint8),
dispatches, and unpacks the spike output.

Per-core device layout ("layout B"): (128 partitions, free) with
partitions = (channel, y-row) packs and free = (x, t), t innermost:
  - input x / psp1 : p = ci*64 + y
  - layers 1..3    : 4 y-groups g; p = co*16 + yl   (y = 16 g + yl)
  - layer 4        : 2 halves h;  p = c*64 + yl'    (y' = 64 h + yl')

Per-layer stages:
  FIR   : truncated alpha-kernel temporal conv == two cascaded 1-pole
          IIRs (tensor_tensor_scan per x-row along t) + exact truncation
          correction (2 bulk fused ops). Output unscaled by nu*b (the
          scale is folded into the downstream conv weight matrices).
  conv  : y-banded matmuls on the PE (contraction over (ci, y-window));
          kx taps = shifted rhs windows into x-padded tensors; PSUM
          accumulation; ACT-engine evac with bias -theta -> u_hat.
  scan  : sequential LIF over t with the exact truncated refractory via
          ring-buffered cascaded IIRs:
            q_t = a q_{t-1} + s_{t-1};  r_t = a r_{t-1} + q_t
            g_t = (L-1) q_{t-L+1} + r_{t-L+1}
            s_t = 1{ (-mu a) r_t <= u_hat_t - mu a^L g_t }   (u8 spikes)
  convT + bilinear-2x upsample of psp1: all PE matmuls accumulated in
  one PSUM (x-interp taps = shifted rhs windows over clamp-padded psp1).

A full host (numpy) implementation is kept as fallback if the device
path is unavailable or fails.
"""

import os

import numpy as np
from numpy.lib.stride_tricks import as_strided

TS = 1.0
T = 128
B, C, H, W = 8, 2, 64, 64
THETA = [30.0, 50.0, 50.0, 100.0]
TAU = [1.0, 2.0, 2.0, 4.0]
CFGS = [(30.0, 1.0, 1.0, 1.0),
        (50.0, 2.0, 2.0, 1.0),
        (50.0, 2.0, 2.0, 1.0),
        (100.0, 4.0, 4.0, 1.0)]


def _alpha_kernel(tau, mult=1.0):
    eps = []
    t = 0.0
    while t < T:
        v = mult * t / tau * np.exp(1.0 - t / tau)
        if abs(v) < abs(mult) * 0.01 and t > tau:
            break
        eps.append(v)
        t += TS
    return np.asarray(eps, np.float32)


LS = [len(_alpha_kernel(t)) for t in TAU]
A = [float(np.exp(-1.0 / t)) for t in TAU]
NU = [float(np.e / t) for t in TAU]
MU = [float(-2.0 * th * np.e / tau) for th, tau in zip(THETA, TAU)]

# ------------------------------------------------------------- device SBUF map
# offsets in f32 elements per partition (208 KiB budget)
OFF_PSP1, LEN_PSP1 = 2560, 68 * 128
OFF_Q = 11264
OFF_R = 21504
OFF_XB = 31744
OFF_U1 = 11264
OFF_GM1 = 0
OFF_S = 44288
OFF_PYG = [0, 31744]
OFF_PS = [0, 8448, 16896]
OFF_UH = 25344
OFF_GM23 = 41728
OFF_P4OUT = [0, 31744]
OFF_P4S = [11264, 19456]
OFF_UH4 = 27648
OFF_GM4 = 0
OFF_RING1 = 0
OFF_RING23 = 0
OFF_RING4 = 11264
OFF_CONST = 52736
ARENA_LEN = 53200

GI_CONV1, GI_CONV2, GI_CONV3, GI_CONVT, GI_UP, N_MATS = 0, 20, 29, 38, 46, 50


def _build_weight_pack(w1, w2, w3, w4):
    """All conv / convT / upsample lhsT matrices (each (K=128, M=128)),
    scaled by the producing FIR's nu*b."""
    mats = []
    s1 = NU[0] * A[0]
    s2 = NU[1] * A[1]
    s4 = NU[3] * A[3]

    for go in range(4):                       # conv1 5x5: [go][kx]
        for kx in range(5):
            g = np.zeros((128, 128), np.float32)
            for co in range(8):
                for yl in range(16):
                    yo = 16 * go + yl
                    for ky in range(5):
                        yi = yo + ky - 2
                        if 0 <= yi < 64:
                            for ci in range(2):
                                g[ci * 64 + yi, co * 16 + yl] = \
                                    w1[co, ci, ky, kx] * s1
            mats.append(g)

    for w in (w2, w3):                        # conv2/3 3x3: mid[3], up[3], dn[3]
        for kx in range(3):
            g = np.zeros((128, 128), np.float32)
            for co in range(8):
                for yl in range(16):
                    for ky in range(3):
                        yi = yl + ky - 1
                        if 0 <= yi < 16:
                            for ci in range(8):
                                g[ci * 16 + yi, co * 16 + yl] = \
                                    w[co, ci, ky, kx] * s2
            mats.append(g)
        for kx in range(3):
            g = np.zeros((128, 128), np.float32)
            for co in range(8):
                for ci in range(8):
                    g[ci * 16 + 15, co * 16 + 0] = w[co, ci, 0, kx] * s2
            mats.append(g)
        for kx in range(3):
            g = np.zeros((128, 128), np.float32)
            for co in range(8):
                for ci in range(8):
                    g[ci * 16 + 0, co * 16 + 15] = w[co, ci, 2, kx] * s2
            mats.append(g)

    for h in range(2):                        # convT 2x2/2: [h][par][dx]
        for par in range(2):
            for dx in range(2):
                g = np.zeros((128, 128), np.float32)
                for c in range(2):
                    for ci in range(8):
                        for yl in range(16):
                            for dy in range(2):
                                m = c * 64 + 32 * par + 2 * yl + dy
                                g[ci * 16 + yl, m] = \
                                    w4[c, ci, 1 - dy, 1 - dx] * s4
                mats.append(g)

    for h in range(2):                        # bilinear-y U75/U25 per half
        uw = np.zeros((64, 64), np.float32)
        for ylp in range(64):
            yp = 64 * h + ylp
            i = yp // 2
            if yp % 2 == 0:
                uw[ylp, i] += 0.75
                uw[ylp, max(i - 1, 0)] += 0.25
            else:
                uw[ylp, i] += 0.75
                uw[ylp, min(i + 1, 63)] += 0.25
        for scale in (0.75, 0.25):
            g = np.zeros((128, 128), np.float32)
            for c in range(2):
                g[c * 64:(c + 1) * 64, c * 64:(c + 1) * 64] = \
                    uw.T * (s1 * scale)
            mats.append(g)

    return np.stack(mats).astype(np.float32)


def _pack_weights_device(w1, w2, w3, w4):
    stack = _build_weight_pack(w1, w2, w3, w4)
    return np.ascontiguousarray(stack.transpose(1, 0, 2).reshape(128, -1))


def _pack_input(x_sample):
    """(2, 64, 64, 128) -> (128, 8192) u8 in (ci*64+y, x*128+t)."""
    return np.ascontiguousarray(
        np.asarray(x_sample).astype(np.uint8).reshape(128, 8192))


def _unpack_output(s4_dev):
    """(128, 2*128*128) u8 -> (2, 128, 128, 128) f32."""
    a = s4_dev.reshape(2, 64, 2, 128, 128)   # [c, yl', h, x', t]
    out = np.empty((2, 128, 128, 128), np.float32)
    for h in range(2):
        out[:, 64 * h:64 * (h + 1)] = a[:, :, h].astype(np.float32)
    return out


def _build_net():
    import concourse.bass as bass
    import concourse.mybir as mybir

    f32 = mybir.dt.float32
    u8 = mybir.dt.uint8
    ALU = mybir.AluOpType
    AF = mybir.ActivationFunctionType

    nc = bass.Bass(detect_race_conditions=False)
    xin = nc.declare_dram_parameter("xin", [128, 64 * 128], u8, isOutput=False)
    wpack = nc.declare_dram_parameter("wpack", [128, N_MATS * 128], f32,
                                      isOutput=False)
    s4out = nc.declare_dram_parameter("s4out", [128, 2 * 128 * 128], u8,
                                      isOutput=True)

    pstash = nc.dram_tensor("pstash", [128, LEN_PSP1], f32, kind="Internal")
    pdram = {1: nc.dram_tensor("pdram2", [128, 4, 66 * 128], f32,
                               kind="Internal"),
             2: nc.dram_tensor("pdram3", [128, 4, 66 * 128], f32,
                               kind="Internal")}
    p4dram = nc.dram_tensor("p4dram", [128, 4, 64 * 128], f32, kind="Internal")

    AR = nc.alloc_sbuf_tensor("AR", [128, ARENA_LEN], f32)
    psum = [nc.alloc_psum_tensor(f"ps{i}", [128, 512], f32) for i in range(8)]

    s_dma = nc.alloc_semaphore("s_dma")
    s_pe = nc.alloc_semaphore("s_pe")
    s_act = nc.alloc_semaphore("s_act")
    s_dve = nc.alloc_semaphore("s_dve")

    V, P, S, Y = nc.vector, nc.tensor, nc.sync, nc.scalar

    class Cnt:
        dma = 0; pe = 0; act = 0; dve = 0

    def ar(off, n):
        return AR[:, off:off + n]

    def dma(dst, src):
        # serialize completions so per-DMA cumulative wait targets are exact
        if Cnt.dma:
            S.wait_ge(s_dma, Cnt.dma)
        Cnt.dma += 16
        S.dma_start(dst, src).then_inc(s_dma, 16)
        return Cnt.dma

    btile = ar(OFF_CONST, 128)
    bias = [ar(OFF_CONST + 128 + i, 1) for i in range(4)]
    ht_flat = ar(OFF_CONST + 160, 256)

    x_rows = AR[:, OFF_XB:OFF_XB + 2048].bitcast(u8).rearrange(
        "p (one x t) -> p one x t", one=1, x=64)
    psp1 = ar(OFF_PSP1, LEN_PSP1).rearrange("p (x t) -> p x t", x=68)
    psp1f = ar(OFF_PSP1, LEN_PSP1)
    qv = ar(OFF_Q, 64 * 160).rearrange("p (x t) -> p x t", x=64)
    rv = ar(OFF_R, 64 * 160).rearrange("p (x t) -> p x t", x=64)
    u1 = ar(OFF_U1, 4 * 64 * 128).rearrange("p (g x t) -> p g x t", g=4, x=64)
    spk = AR[:, OFF_S:OFF_S + 8192].bitcast(u8)
    s123 = spk.rearrange("p (g x t) -> p g x t", g=4, x=64)
    s4v = spk.rearrange("p (h x t) -> p h x t", h=2, x=128)
    uh = ar(OFF_UH, 2 * 64 * 128).rearrange("p (g x t) -> p g x t", g=2, x=64)
    uh4 = ar(OFF_UH4, 128 * 128).rearrange("p (x two t) -> p x two t",
                                           two=2, t=T)

    def fir_phase(li, in_view, n_groups, out_slots, out_xpad, dram_dst):
        """Truncated FIR per y-group: per-x-row IIR cascades + combine."""
        b, Ls = A[li], LS[li]
        cL = float(b ** (Ls - 1))
        if Cnt.dma:
            V.wait_ge(s_dma, Cnt.dma)
        V.memset(btile, b)
        V.memset(qv[:, :, 0:32], 0.0)
        m = V.memset(rv[:, :, 0:32], 0.0)
        Cnt.dve += 1
        m.then_inc(s_dve, 1)
        slot_dma = {}
        out_targets = []
        for g in range(n_groups):
            for x in range(64):
                V.tensor_tensor_scan(qv[:, x, 32:160], btile,
                                     in_view[:, g, x, :], 0.0,
                                     ALU.mult, ALU.add)
                V.tensor_tensor_scan(rv[:, x, 32:160], btile,
                                     qv[:, x, 32:160], 0.0,
                                     ALU.mult, ALU.add)
            if dram_dst is None:
                out_ap = psp1[:, 2:66, :]
            else:
                slot = out_slots[g % 2]
                Wd = out_xpad + 64 + out_xpad
                pygv = ar(slot, Wd * 128).rearrange("p (x t) -> p x t", x=Wd)
                out_ap = pygv[:, out_xpad:out_xpad + 64, :]
                if g < 2 and out_xpad:
                    V.memset(pygv[:, 0:out_xpad, :], 0.0)
                    V.memset(pygv[:, out_xpad + 64:, :], 0.0)
                if g >= 2:
                    V.wait_ge(s_dma, slot_dma[g % 2])
            V.scalar_tensor_tensor(out_ap, qv[:, :, 32 - Ls:160 - Ls],
                                   -(Ls - 1.0) * cL, rv[:, :, 31:159],
                                   ALU.mult, ALU.add)
            cmb = V.scalar_tensor_tensor(out_ap, rv[:, :, 32 - Ls:160 - Ls],
                                         -cL, out_ap, ALU.mult, ALU.add)
            Cnt.dve += 1
            cmb.then_inc(s_dve, 1)
            if dram_dst is not None:
                S.wait_ge(s_dve, Cnt.dve)
                Wd = out_xpad + 64 + out_xpad
                tgt = dma(dram_dst[:, g, :], ar(out_slots[g % 2], Wd * 128))
                slot_dma[g % 2] = tgt
                out_targets.append(tgt)
        return out_targets, Cnt.dve

    def scan_phase(li, u_view, s_view, ng, X, ring_off, act_target):
        a, L, mu = A[li], LS[li], MU[li]
        c6 = float(mu * (A[li] ** L))
        c7 = float(-mu * a)
        R = L + 6
        FD = ng * X
        qr_ = ar(ring_off, R * FD).rearrange("p (s f) -> p s f", s=R)
        rr_ = ar(ring_off + R * FD, R * FD).rearrange("p (s f) -> p s f", s=R)
        gr_ = ar(ring_off + 2 * R * FD, FD)
        ht = ht_flat[:, 0:FD]

        def shp(apx):
            return apx.rearrange("p (g x) -> p g x", g=ng)

        V.wait_ge(s_act, act_target)
        if Cnt.dma:
            V.wait_ge(s_dma, Cnt.dma)
        V.memset(qr_[:, 0], 0.0)
        V.memset(rr_[:, 0], 0.0)
        o7 = None
        for t in range(T):
            if t >= 1:
                V.scalar_tensor_tensor(shp(qr_[:, t % R]),
                                       shp(qr_[:, (t - 1) % R]), a,
                                       s_view[:, :, :, t - 1],
                                       ALU.mult, ALU.add)
                V.scalar_tensor_tensor(rr_[:, t % R], rr_[:, (t - 1) % R], a,
                                       qr_[:, t % R], ALU.mult, ALU.add)
            if t >= L:
                tau = (t - L + 1) % R
                V.scalar_tensor_tensor(gr_, qr_[:, tau], float(L - 1),
                                       rr_[:, tau], ALU.mult, ALU.add)
                V.scalar_tensor_tensor(shp(ht), shp(gr_), -c6,
                                       u_view[:, :, :, t], ALU.mult, ALU.add)
                in1 = shp(ht)
            else:
                in1 = u_view[:, :, :, t]
            o7 = V.scalar_tensor_tensor(s_view[:, :, :, t],
                                        shp(rr_[:, t % R]), c7, in1,
                                        ALU.mult, ALU.is_le)
        Cnt.dve += 1
        o7.then_inc(s_dve, 1)
        return Cnt.dve

    class Psum:
        j = 0
        targets = {}

    def psum_group():
        jj = Psum.j
        Psum.j += 1
        if jj >= 8:
            P.wait_ge(s_act, Psum.targets[jj - 8])
        return jj, psum[jj % 8]

    def psum_done(jj):
        Cnt.act += 1
        Psum.targets[jj] = Cnt.act

    def conv_group(bank, mats_rhs):
        last = None
        n = len(mats_rhs)
        for i, (lhs, rhs) in enumerate(mats_rhs):
            last = P.matmul(bank[:], lhs, rhs, start=(i == 0),
                            stop=(i == n - 1))
        return last

    gm1 = ar(OFF_GM1, 20 * 128).rearrange("p (m k) -> p m k", m=20)
    gm23 = ar(OFF_GM23, 9 * 128).rearrange("p (m k) -> p m k", m=9)
    gm4 = ar(OFF_GM4, 12 * 128).rearrange("p (m k) -> p m k", m=12)

    # ---------------- P0: input + FIR1 -> psp1
    dma_x = dma(AR[:, OFF_XB:OFF_XB + 2048].bitcast(u8), xin[:])
    V.wait_ge(s_dma, dma_x)
    for i, th in enumerate(THETA):
        V.memset(bias[i], -th)
    V.memset(psp1[:, 0:2, :], 0.0)
    V.memset(psp1[:, 66:68, :], 0.0)
    _, fir1_done = fir_phase(0, x_rows, 1, None, 0, None)

    # ---------------- P1: conv1 -> u1_hat, stash psp1
    tgt_gm1 = dma(ar(OFF_GM1, 20 * 128),
                  wpack[:, GI_CONV1 * 128:(GI_CONV1 + 20) * 128])
    P.wait_ge(s_dma, tgt_gm1)
    P.wait_ge(s_dve, fir1_done)
    for go in range(4):
        for xc in range(16):
            jj, bank = psum_group()
            mr = []
            for kx in range(5):
                s0 = (4 * xc + kx) * 128
                mr.append((gm1[:, go * 5 + kx, :], psp1f[:, s0:s0 + 512]))
            stop = conv_group(bank, mr)
            Cnt.pe += 1
            stop.then_inc(s_pe, 1)
            Y.wait_ge(s_pe, Cnt.pe)
            ev = Y.activation(u1[:, go, 4 * xc:4 * xc + 4, :], bank[:],
                              AF.Identity, bias=bias[0], scale=1.0)
            psum_done(jj)
            ev.then_inc(s_act, 1)
    conv1_acts = Cnt.act
    S.wait_ge(s_pe, Cnt.pe)
    dma(pstash[:], psp1f)

    # ---------------- P2: scan1
    scan_phase(0, u1, s123, 4, 64, OFF_RING1, conv1_acts)

    # ---------------- P3..P6: layers 2 and 3
    def layer23(li):
        pd = pdram[li]
        out_tgts, _ = fir_phase(li, s123, 4, OFF_PYG, 1, pd)
        fir_dma = Cnt.dma
        gi_base = GI_CONV2 if li == 1 else GI_CONV3
        tgt_gm = dma(ar(OFF_GM23, 9 * 128),
                     wpack[:, gi_base * 128:(gi_base + 9) * 128])
        load = {}
        ps_slot = {}
        for gi in range(3):
            S.wait_ge(s_dma, out_tgts[gi])
            load[gi] = dma(ar(OFF_PS[gi], 66 * 128), pd[:, gi, :])
            ps_slot[gi] = gi
        for half in range(2):
            if half == 1:
                S.wait_ge(s_dve, Cnt.dve)      # scan-A rings dead
                S.wait_ge(s_dma, out_tgts[3])
                load[3] = dma(ar(OFF_PS[0], 66 * 128), pd[:, 3, :])
                ps_slot[3] = 0
            for go in ((0, 1) if half == 0 else (2, 3)):
                gis = [gi for gi in (go - 1, go, go + 1) if 0 <= gi < 4]
                P.wait_ge(s_dma, max(max(load[gi] for gi in gis), tgt_gm))
                for xc in range(16):
                    jj, bank = psum_group()
                    mr = []
                    for gi in gis:
                        rel = gi - go
                        pf = ar(OFF_PS[ps_slot[gi]], 66 * 128)
                        for kx in range(3):
                            mi = kx if rel == 0 else (3 + kx if rel == -1
                                                      else 6 + kx)
                            s0 = (4 * xc + kx) * 128
                            mr.append((gm23[:, mi, :], pf[:, s0:s0 + 512]))
                    stop = conv_group(bank, mr)
                    Cnt.pe += 1
                    stop.then_inc(s_pe, 1)
                    Y.wait_ge(s_pe, Cnt.pe)
                    if half == 0 and go == 0 and xc == 0:
                        Y.wait_ge(s_dma, fir_dma)  # WAR: uh over pygB dma-out
                    ev = Y.activation(uh[:, go % 2, 4 * xc:4 * xc + 4, :],
                                      bank[:], AF.Identity, bias=bias[li],
                                      scale=1.0)
                    psum_done(jj)
                    ev.then_inc(s_act, 1)
            shalf = s123[:, 2 * half:2 * half + 2]
            scan_phase(li, uh, shalf, 2, 64, OFF_RING23, Cnt.act)

    layer23(1)
    layer23(2)

    # ---------------- P7: FIR4 -> p4dram
    _, _ = fir_phase(3, s123, 4, OFF_P4OUT, 0, p4dram)
    fir4_dma = Cnt.dma

    # ---------------- P8: convT + upsample + scan4
    S.wait_ge(s_dma, fir4_dma)
    tgt_gm4 = dma(ar(OFF_GM4, 12 * 128),
                  wpack[:, GI_CONVT * 128:(GI_CONVT + 12) * 128])
    tgt_psp = dma(psp1f, pstash[:])
    V.wait_ge(s_dma, tgt_psp)
    V.tensor_copy(psp1[:, 1, :], psp1[:, 2, :])
    cp = V.tensor_copy(psp1[:, 66, :], psp1[:, 65, :])
    Cnt.dve += 1
    cp.then_inc(s_dve, 1)
    clamp_done = Cnt.dve
    for h in range(2):
        if h == 1:
            S.wait_ge(s_dve, Cnt.dve)          # scan4-0 rings dead
        p4load = {}
        for par in range(2):
            p4load[par] = dma(ar(OFF_P4S[par], 64 * 128),
                              p4dram[:, 2 * h + par, :])
        P.wait_ge(s_dma, max(tgt_gm4, p4load[0], p4load[1]))
        P.wait_ge(s_dve, clamp_done)
        for dx in range(2):
            for xc in range(16):
                jj, bank = psum_group()
                mr = []
                for par in range(2):
                    pf = ar(OFF_P4S[par], 64 * 128)
                    mr.append((gm4[:, h * 4 + par * 2 + dx, :],
                               pf[:, 512 * xc:512 * (xc + 1)]))
                u75 = gm4[:, 8 + 2 * h, :]
                u25 = gm4[:, 8 + 2 * h + 1, :]
                b0 = (2 + 4 * xc) * 128
                sh = -128 if dx == 0 else 128
                mr.append((u75, psp1f[:, b0:b0 + 512]))
                mr.append((u25, psp1f[:, b0 + sh:b0 + sh + 512]))
                stop = conv_group(bank, mr)
                Cnt.pe += 1
                stop.then_inc(s_pe, 1)
                Y.wait_ge(s_pe, Cnt.pe)
                ev = Y.activation(uh4[:, 4 * xc:4 * xc + 4, dx, :], bank[:],
                                  AF.Identity, bias=bias[3], scale=1.0)
                psum_done(jj)
                ev.then_inc(s_act, 1)
        uview4 = ar(OFF_UH4, 128 * 128).rearrange(
            "p (one x t) -> p one x t", one=1, x=128)
        scan_phase(3, uview4, s4v[:, h:h + 1], 1, 128, OFF_RING4, Cnt.act)

    # ---------------- P9: output
    S.wait_ge(s_dve, Cnt.dve)
    dma(s4out[:], spk[:, :])
    return nc


_STATE = {}


def _get_runner():
    """Persistent jitted shard_map runner over the 8 cores (compiles once)."""
    if "runner" in _STATE:
        return _STATE["runner"]
    import jax
    from concourse import bass2jax, mybir
    from jax.sharding import Mesh, NamedSharding, PartitionSpec
    from jax.experimental.shard_map import shard_map

    nc = _build_net()
    bass2jax.install_neuronx_cc_hook()
    partition_name = (nc.partition_id_tensor.name
                      if nc.partition_id_tensor else None)
    in_names, out_names, out_avals, zero_shapes = [], [], [], []
    for alloc in nc.m.functions[0].allocations:
        if not isinstance(alloc, mybir.MemoryLocationSet):
            continue
        name = alloc.memorylocations[0].name
        if alloc.kind == "ExternalInput":
            if name != partition_name:
                in_names.append(name)
        elif alloc.kind == "ExternalOutput":
            out_names.append(name)
            shape = tuple(alloc.tensor_shape)
            dtype = mybir.dt.np(alloc.dtype)
            out_avals.append(jax.core.ShapedArray(shape, dtype))
            zero_shapes.append((shape, dtype))
    n_params = len(in_names)
    all_names = list(in_names) + list(out_names) + (
        [partition_name] if partition_name else [])

    def _body(*args):
        operands = list(args)
        if partition_name is not None:
            operands.append(bass2jax.partition_id_tensor())
        outs = bass2jax._bass_exec_p.bind(
            *operands, out_avals=tuple(out_avals), in_names=tuple(all_names),
            out_names=tuple(out_names), lowering_input_output_aliases=(),
            sim_require_finite=False, sim_require_nnan=False, nc=nc)
        return tuple(outs)

    devices = jax.devices()[:B]
    mesh = Mesh(np.asarray(devices), ("core",))
    sharded = jax.jit(
        shard_map(_body, mesh=mesh,
                  in_specs=(PartitionSpec("core"),) * (n_params + len(out_avals)),
                  out_specs=(PartitionSpec("core"),) * len(out_avals),
                  check_rep=False),
        keep_unused=True)
    sh = NamedSharding(mesh, PartitionSpec("core"))
    runner = {"sharded": sharded, "in_names": in_names,
              "out_names": out_names, "zero_shapes": zero_shapes,
              "sharding": sh, "jax": jax}
    _STATE["runner"] = runner
    return runner


def _device_forward(spikeInput, w1, w2, w3, w4):
    rn = _get_runner()
    jax = rn["jax"]
    wp = _pack_weights_device(w1, w2, w3, w4)
    per_core = {"xin": np.stack([_pack_input(spikeInput[i]) for i in range(B)]),
                "wpack": np.stack([wp] * B)}
    concat_in = [per_core[n].reshape(B * per_core[n].shape[1],
                                    *per_core[n].shape[2:])
                 for n in rn["in_names"]]
    zeros = [np.zeros((B * s[0], *s[1:]), d) for s, d in rn["zero_shapes"]]
    outs = rn["sharded"](*concat_in, *zeros)
    jax.block_until_ready(outs)
    i4 = rn["out_names"].index("s4out")
    s4 = np.asarray(outs[i4]).reshape(B, 128, 2 * 128 * 128)
    full = np.stack([_unpack_output(s4[i]) for i in range(B)])
    if not np.all((full == 0.0) | (full == 1.0)):
        raise RuntimeError("device produced non-binary spikes")
    return full


def _axon_devices_visible():
    try:
        import jax

        return sum(1 for d in jax.devices() if d.platform != "cpu")
    except Exception:
        return 0


# ------------------------------------------------------------- host fallback
_SRM = [_alpha_kernel(c[1]) for c in CFGS]
_REFK = [_alpha_kernel(c[2], mult=-2.0 * c[0] * c[3]) for c in CFGS]


def _psp_mat(k):
    m = np.zeros((T, T), np.float32)
    for j in range(len(k)):
        if k[j] != 0.0:
            m += np.diag(np.full(T - j, k[j], np.float32), k=j)
    return m


_PSP_M = [_psp_mat(s) for s in _SRM]


def _psp_blas(x, li):
    if not x.any():
        return np.zeros_like(x)
    sh = x.shape
    return (x.reshape(-1, sh[-1]) @ _PSP_M[li]).reshape(sh)


def _conv2d_fast(x, w, pad, cb_in=False, cb_out=False):
    if cb_in:
        c, b, h, ww, t = x.shape
    else:
        b, c, h, ww, t = x.shape
    co, ci, kh, kw = w.shape
    if not x.any():
        oshape = (co, b, h, ww, t) if cb_out else (b, co, h, ww, t)
        return np.zeros(oshape, np.float32)
    xp = np.zeros((ci, b, h + 2 * pad, ww + 2 * pad, t), np.float32)
    xp[:, :, pad:pad + h, pad:pad + ww] = x if cb_in else \
        x.transpose(1, 0, 2, 3, 4)
    if ci >= 8:
        Hp, Wp = h + 2 * pad, ww + 2 * pad
        xf = xp.reshape(ci, -1)
        n = xf.shape[1]
        row = Wp * t
        out_pad = np.zeros((co, n), np.float32)
        tmp = np.empty((co, n), np.float32)
        first = True
        for ky in range(kh):
            for kx in range(kw):
                off = (ky - pad) * row + (kx - pad) * t
                wk = np.ascontiguousarray(w[:, :, ky, kx])
                if off >= 0:
                    np.matmul(wk, xf[:, off:], out=tmp[:, :n - off])
                    if first:
                        out_pad[:, :n - off] = tmp[:, :n - off]
                    else:
                        out_pad[:, :n - off] += tmp[:, :n - off]
                else:
                    np.matmul(wk, xf[:, :n + off], out=tmp[:, -off:])
                    if first:
                        out_pad[:, -off:] = tmp[:, -off:]
                    else:
                        out_pad[:, -off:] += tmp[:, -off:]
                first = False
        out = out_pad.reshape(co, b, Hp, Wp, t)[:, :, pad:pad + h,
                                                pad:pad + ww, :]
    else:
        sC, sB, sH, sW, sT = xp.strides
        v = as_strided(xp, shape=(ci, kh, kw, b, h, ww, t),
                       strides=(sC, sH, sW, sB, sH, sW, sT))
        out = np.tensordot(w, v, axes=([1, 2, 3], [0, 1, 2]))
    if cb_out:
        return np.ascontiguousarray(out)
    return np.ascontiguousarray(out.transpose(1, 0, 2, 3, 4))


def _convT2d_fast(x, w, cb_in=False):
    if cb_in:
        c, b, h, ww, t = x.shape
    else:
        b, c, h, ww, t = x.shape
    co, ci = w.shape[0], w.shape[1]
    out = np.zeros((b, co, 2 * h, 2 * ww, t), np.float32)
    if not x.any():
        return out
    xt = (x if cb_in else
          np.ascontiguousarray(x.transpose(1, 0, 2, 3, 4))).reshape(ci, -1)
    for dy in range(2):
        for dx in range(2):
            wk = np.ascontiguousarray(w[:, :, 1 - dy, 1 - dx])
            r = (wk @ xt).reshape(co, b, h, ww, t)
            out[:, :, dy::2, dx::2, :] = r.transpose(1, 0, 2, 3, 4)
    return out


def _up2_axis(a, axis):
    a = np.moveaxis(a, axis, 0)
    c75 = np.float32(0.75)
    c25 = np.float32(0.25)
    out = np.empty((2 * a.shape[0],) + a.shape[1:], np.float32)
    ev = out[0::2]
    od = out[1::2]
    ev[1:] = c75 * a[1:] + c25 * a[:-1]
    ev[0] = c75 * a[0] + c25 * a[0]
    od[:-1] = c75 * a[:-1] + c25 * a[1:]
    od[-1] = c75 * a[-1] + c25 * a[-1]
    return np.moveaxis(out, 0, axis)


def _upsample2_np(x):
    return _up2_axis(_up2_axis(x, 2), 3)


def _spike_scan_iir(u, li):
    theta, _, tau, scale_ref = CFGS[li]
    refk = _REFK[li]
    L = len(refk)
    mult = -2.0 * theta * scale_ref
    a = np.exp(-1.0 / tau)
    a2 = np.float32(a * a)
    two_a = np.float32(2.0 * a)
    c1 = np.float32(refk[1])
    cLv = np.float32(mult * (L / tau) * np.exp(1.0 - L / tau))
    cLm1 = np.float32(a * a * refk[L - 1])
    th = np.float32(theta)

    sh = u.shape
    t_n = sh[-1]
    un0 = u.reshape(-1, t_n)
    n0 = un0.shape[0]
    act = (un0 >= th).any(axis=1)
    s_full = np.zeros((n0, t_n), np.float32)
    if not act.any():
        return s_full.reshape(sh)
    idx = np.nonzero(act)[0]
    un = np.ascontiguousarray(un0[idx])
    n = un.shape[0]
    s = np.zeros((n, t_n), np.float32)
    cnt = np.zeros(t_n + 1, np.int64)
    d1 = np.zeros(n, np.float32)
    d2 = np.zeros(n, np.float32)
    d = np.empty(n, np.float32)
    ue = np.empty(n, np.float32)
    dirty = False
    last_spike = -(10 ** 9)
    for t in range(t_n):
        if dirty and t - last_spike > L + 1:
            d1[:] = 0.0
            d2[:] = 0.0
            dirty = False
        inj = (t >= 1 and cnt[t - 1]) or (t >= L and cnt[t - L]) \
            or (t >= L + 1 and cnt[t - L - 1])
        if dirty or inj:
            np.multiply(d1, two_a, out=d)
            d -= a2 * d2
            if t >= 1 and cnt[t - 1]:
                d += c1 * s[:, t - 1]
            if t >= L and cnt[t - L]:
                d -= cLv * s[:, t - L]
            if t >= L + 1 and cnt[t - L - 1]:
                d += cLm1 * s[:, t - L - 1]
            np.add(un[:, t], d, out=ue)
            d2, d1, d = d1, d, d2
            dirty = True
            st = ue >= th
        else:
            st = un[:, t] >= th
        c = int(np.count_nonzero(st))
        cnt[t] = c
        if c:
            s[:, t] = st
            last_spike = t
    s_full[idx] = s
    return s_full.reshape(sh)


def _host_forward(spikeInput, w1, w2, w3, w4):
    psp1 = _psp_blas(spikeInput, 0)
    psp1_up = _upsample2_np(psp1)
    s1 = _spike_scan_iir(_conv2d_fast(psp1, w1, 2, cb_out=True), 0)
    s2 = _spike_scan_iir(
        _conv2d_fast(_psp_blas(s1, 1), w2, 1, cb_in=True, cb_out=True), 1)
    s3 = _spike_scan_iir(
        _conv2d_fast(_psp_blas(s2, 2), w3, 1, cb_in=True, cb_out=True), 2)
    u4 = _convT2d_fast(_psp_blas(s3, 3), w4, cb_in=True) + psp1_up
    s4 = _spike_scan_iir(u4, 3)
    return s4.astype(np.float32)


def kernel(spikeInput, w1, w2, w3, w4):
    spikeInput = np.ascontiguousarray(np.asarray(spikeInput, np.float32))
    w1 = np.asarray(w1, np.float32)
    w2 = np.asarray(w2, np.float32)
    w3 = np.asarray(w3, np.float32)
    w4 = np.asarray(w4, np.float32)

    if os.environ.get("NN1_FORCE_HOST") != "1" and _axon_devices_visible() >= B:
        try:
            return _device_forward(spikeInput, w1, w2, w3, w4)
        except Exception:
            pass
    return _host_forward(spikeInput, w1, w2, w3, w4)
